# revision 1
# baseline (speedup 1.0000x reference)
"""GRU decoder (nn_Decoder) on 8 TRN2 NeuronCores.

Strategy:
- Host precompute (index-only / transposes / casts): fed tokens x_t are known
  (x_0=SOS, x_t=target[t-1]); gather embeddings E_all [2048,512] on host,
  precompute done masks, transpose+cast weights to bf16.
- Phase 0 (device): gi = E_all @ w_ih_slice.T for this core's 3x128 gate rows,
  batched over all 64 steps.
- Phase A (device): 64-step recurrence, h sharded 8 ways (core c owns h cols
  [128c,128c+128) and the matching 3 gate row slices). Per step: 8 accumulating
  matmuls [32,384] + gate nonlinearities + per-step all-gather of the bf16
  h^T slice [128,32] to every core.
- Phase B (device, interleaved): vocab-sharded output projection with rows
  (step,batch) on partitions: out[128 rows, 4000 vocab] per m-chunk, softmax
  sums as free-dim reduces, cross-core AllReduce of denominators batched every
  4 m-chunks, done-masking folded into the normalization scale.
"""

import numpy as np
import ml_dtypes

VOCAB = 32000
EMB = 512
HID = 1024
B = 32
SEQ = 64
PAD_ID = 0
SOS_ID = 1
EOS_ID = 2
NC = 8
HSL = HID // NC          # 128 h columns per core
GS = 3 * HSL             # 384 gate rows per core
VS = VOCAB // NC         # 4000 vocab per core
ROWS = SEQ * B           # 2048
MCH = ROWS // 128        # 16 m-chunks (4 steps each)
ARB = 4                  # m-chunks per AllReduce batch
VT = 8                   # vocab tiles per m-chunk (4000 = 8*500)
VTW = VS // VT           # 500

BF16 = ml_dtypes.bfloat16

_CACHE = {}


def _build(exchange="ag"):
    import concourse.bass as bass
    import concourse.mybir as mybir
    import concourse.tile as tile
    from concourse import bacc
    from concourse.masks import make_identity

    f32 = mybir.dt.float32
    bf16 = mybir.dt.bfloat16
    AF = mybir.ActivationFunctionType
    ALU = mybir.AluOpType
    AX = mybir.AxisListType

    nc = bacc.Bacc(None, target_bir_lowering=False, num_devices=NC)

    # ---- kernel I/O (per-core shards prepared by host) ----
    eT_d = nc.dram_tensor("eT", [EMB, ROWS], bf16, kind="ExternalInput")
    wihT_d = nc.dram_tensor("wihT", [EMB, GS], bf16, kind="ExternalInput")
    whhT_d = nc.dram_tensor("whhT", [HID, GS], bf16, kind="ExternalInput")
    woutT_d = nc.dram_tensor("woutT", [HID, VS], bf16, kind="ExternalInput")
    hT0_d = nc.dram_tensor("hT0", [HID, B], bf16, kind="ExternalInput")
    h0own_d = nc.dram_tensor("h0own", [B, HSL], f32, kind="ExternalInput")
    done_d = nc.dram_tensor("doneS", [B, SEQ], f32, kind="ExternalInput")
    live_d = nc.dram_tensor("livePB", [128, MCH], f32, kind="ExternalInput")
    pad_d = nc.dram_tensor("padPB", [128, MCH], f32, kind="ExternalInput")
    out_d = nc.dram_tensor("out", [ROWS, VS], f32, kind="ExternalOutput")

    with tile.TileContext(nc) as tc:
        with (
            tc.tile_pool(name="wts", bufs=1) as wts,
            tc.tile_pool(name="state", bufs=1) as state,
            tc.tile_pool(name="hown", bufs=2) as hown_pool,
            tc.tile_pool(name="gtmp", bufs=3) as gtmp,
            tc.tile_pool(name="pgh", bufs=2, space="PSUM") as pgh_pool,
            tc.tile_pool(name="ptr", bufs=1, space="PSUM") as ptr_pool,
            tc.tile_pool(name="ppb", bufs=3, space="PSUM") as ppb_pool,
            tc.tile_pool(name="pgi32", bufs=2, space="PSUM") as pgi32_pool,
            tc.tile_pool(name="dram", bufs=4, space="DRAM") as dram,
            tc.tile_pool(name="dram2", bufs=2, space="DRAM") as dram2,
        ):
            # ---- resident tiles ----
            whhT = wts.tile([128, HID // 128, GS], bf16)       # 0.79 MB
            woutT = wts.tile([128, HID // 128, VS], bf16)      # 8.2 MB
            gi = state.tile([128, MCH, GS], bf16)              # 1.6 MB
            hT = state.tile([128, NC, SEQ + 1, B], bf16)       # 4.26 MB
            doneS = state.tile([B, SEQ], f32)
            livePB = state.tile([128, MCH], f32)
            padPB = state.tile([128, MCH], f32)
            ident = state.tile([128, 128], bf16)
            sums = state.tile([128, MCH], f32)
            denoms = state.tile([128, MCH], f32)

            make_identity(nc, ident[:])

            nc.sync.dma_start(whhT[:], whhT_d.rearrange("(c p) n -> p c n", p=128))
            nc.sync.dma_start(woutT[:], woutT_d.rearrange("(c p) n -> p c n", p=128))
            nc.sync.dma_start(hT[:, :, 0, :], hT0_d.rearrange("(c p) n -> p c n", p=128))
            nc.sync.dma_start(doneS[:], done_d[:])
            nc.sync.dma_start(livePB[:], live_d[:])
            nc.sync.dma_start(padPB[:], pad_d[:])

            h_own_init = hown_pool.tile([B, HSL], f32, name="h_own")
            nc.sync.dma_start(h_own_init[:], h0own_d[:])
            h_own = h_own_init

            # ---- phase 0: gi[m] = (E @ w_ih_slice.T)[128 rows] ----
            with tc.tile_pool(name="ph0", bufs=1) as ph0:
                eT = ph0.tile([128, EMB // 128, ROWS], bf16)   # 2.1 MB
                wihT = ph0.tile([128, EMB // 128, GS], bf16)   # 0.39 MB
                nc.sync.dma_start(eT[:], eT_d.rearrange("(c p) n -> p c n", p=128))
                nc.sync.dma_start(wihT[:], wihT_d.rearrange("(c p) n -> p c n", p=128))
                for m in range(MCH):
                    pgi = pgi32_pool.tile([128, GS], f32, name="pgi32")
                    for k in range(EMB // 128):
                        nc.tensor.matmul(
                            pgi[:],
                            eT[:, k, bass.ts(m, 128)],
                            wihT[:, k, :],
                            start=(k == 0),
                            stop=(k == EMB // 128 - 1),
                        )
                    nc.vector.tensor_copy(gi[:, m, :], pgi[:])

            expp_ctx = tc.tile_pool(name="expp", bufs=ARB + 1)
            outp_ctx = tc.tile_pool(name="outp", bufs=2)
            expp = expp_ctx.__enter__()
            outp = outp_ctx.__enter__()

            # ---- helpers ----
            def exchange_ag(src_sb, t):
                """All-gather this core's h^T slice [128,32] into hT[:, :, t+1, :]."""
                agin = dram.tile([128, B], bf16, name="agin")
                agout = dram2.tile([NC * 128, B], bf16, name="agout")
                nc.gpsimd.dma_start(agin[:], src_sb[:])
                nc.gpsimd.collective_compute(
                    "AllGather",
                    mybir.AluOpType.bypass,
                    replica_groups=[list(range(NC))],
                    ins=[agin.opt()],
                    outs=[agout.opt()],
                )
                nc.gpsimd.dma_start(
                    hT[:, :, t + 1, :],
                    agout.rearrange("(c p) n -> p c n", p=128),
                )

            # ---- phase A single step ----
            def step(t):
                nonlocal h_own
                m, po = t // 4, (t % 4) * B
                # extract gi rows [po:po+32] of chunk m down to partitions 0-31
                # via PE: out = ident[:, po:po+32].T @ gi[:, m, :]
                pgi32 = pgi32_pool.tile([B, GS], f32, name="pgi32")
                nc.tensor.matmul(pgi32[:], ident[:, po:po + B], gi[:, m, :],
                                 start=True, stop=True)
                gi_t = gtmp.tile([B, GS], f32, name="gi_c")
                nc.vector.tensor_copy(gi_t[:], pgi32[:])
                pgh = pgh_pool.tile([B, GS], f32, name="pgh")
                for k in range(NC):
                    nc.tensor.matmul(
                        pgh[:],
                        hT[:, k, t, :],
                        whhT[:, k, :],
                        start=(k == 0),
                        stop=(k == NC - 1),
                    )
                rz_in = gtmp.tile([B, 2 * HSL], f32, name="rz_in")
                nc.vector.tensor_tensor(rz_in[:], pgh[:, :2 * HSL], gi_t[:, :2 * HSL], ALU.add)
                rz = gtmp.tile([B, 2 * HSL], f32, name="rz")
                nc.scalar.activation(rz[:], rz_in[:], AF.Sigmoid)
                t1 = gtmp.tile([B, HSL], f32, name="t1")
                nc.vector.tensor_tensor(t1[:], rz[:, :HSL], pgh[:, 2 * HSL:], ALU.mult)
                nc.vector.tensor_tensor(t1[:], t1[:], gi_t[:, 2 * HSL:], ALU.add)
                n_t = gtmp.tile([B, HSL], f32, name="n_t")
                nc.scalar.activation(n_t[:], t1[:], AF.Tanh)
                zp = gtmp.tile([B, HSL], f32, name="zp")
                nc.vector.tensor_scalar(zp[:], rz[:, HSL:], doneS[:, t:t + 1], None, ALU.max)
                d_t = gtmp.tile([B, HSL], f32, name="d_t")
                nc.vector.tensor_tensor(d_t[:], n_t[:], h_own[:], ALU.subtract)
                nc.vector.tensor_tensor(d_t[:], zp[:], d_t[:], ALU.mult)
                h_new = hown_pool.tile([B, HSL], f32, name="h_own")
                nc.vector.tensor_tensor(h_new[:], n_t[:], d_t[:], ALU.subtract)
                h_own = h_new
                hbf = gtmp.tile([B, HSL], bf16, name="hbf")
                nc.vector.tensor_copy(hbf[:], h_new[:])
                ptr = ptr_pool.tile([HSL, B], bf16, name="ptr")
                nc.tensor.transpose(ptr[:], hbf[:], ident[:B, :B])
                src = gtmp.tile([HSL, B], bf16, name="src")
                nc.vector.tensor_copy(src[:], ptr[:])
                exchange_ag(src, t)

            # ---- phase B m-chunk (rows 128m..128m+128 = steps 4m..4m+3) ----
            def pb_mm(m):
                expb = expp.tile([128, VS], bf16, name="expb")
                for v in range(VT):
                    ppb = ppb_pool.tile([128, VTW], f32, name="ppb")
                    for k in range(NC):
                        nc.tensor.matmul(
                            ppb[:],
                            hT[:, k, 4 * m + 1:4 * m + 5, :],
                            woutT[:, k, bass.ts(v, VTW)],
                            start=(k == 0),
                            stop=(k == NC - 1),
                        )
                    nc.scalar.activation(expb[:, bass.ts(v, VTW)], ppb[:], AF.Exp)
                    s_v = gtmp.tile([128, 1], f32, name="s_v")
                    nc.vector.reduce_sum(s_v[:], expb[:, bass.ts(v, VTW)], AX.X)
                    if v == 0:
                        nc.vector.tensor_copy(sums[:, m:m + 1], s_v[:])
                    else:
                        nc.vector.tensor_tensor(sums[:, m:m + 1], sums[:, m:m + 1], s_v[:], ALU.add)
                return expb

            def pb_allreduce(m_hi, nb):
                """AllReduce sums for m-chunks [m_hi-nb+1 .. m_hi]."""
                m_lo = m_hi - nb + 1
                arin = dram.tile([128, nb], f32, name="arin")
                arout = dram2.tile([128, nb], f32, name="arout")
                nc.gpsimd.dma_start(arin[:], sums[:, m_lo:m_hi + 1])
                nc.gpsimd.collective_compute(
                    "AllReduce",
                    mybir.AluOpType.add,
                    replica_groups=[list(range(NC))],
                    ins=[arin.opt()],
                    outs=[arout.opt()],
                )
                nc.gpsimd.dma_start(denoms[:, m_lo:m_hi + 1], arout[:])

            def pb_norm(m, expb):
                inv = gtmp.tile([128, 1], f32, name="inv")
                nc.vector.reciprocal(inv[:], denoms[:, m:m + 1])
                sc = gtmp.tile([128, 1], f32, name="sc")
                nc.vector.tensor_tensor(sc[:], livePB[:, m:m + 1], inv[:], ALU.mult)
                ouf = outp.tile([128, VS], f32, name="ouf")
                nc.vector.tensor_scalar(ouf[:], expb[:], sc[:], None, ALU.mult)
                nc.vector.tensor_tensor(ouf[:, 0:1], ouf[:, 0:1], padPB[:, m:m + 1], ALU.add)
                nc.sync.dma_start(out_d[bass.ts(m, 128), :], ouf[:])

            # ---- main interleaved schedule ----
            pending = []  # (m, expb) awaiting denominators
            for t in range(SEQ):
                step(t)
                if t % 4 == 3:
                    m = t // 4
                    expb = pb_mm(m)
                    pending.append((m, expb))
                    if m % ARB == ARB - 1:
                        pb_allreduce(m, ARB)
                        for pm, pe in pending:
                            pb_norm(pm, pe)
                        pending = []
            outp_ctx.__exit__(None, None, None)
            expp_ctx.__exit__(None, None, None)

    nc.compile()
    return nc


def _host_prep(hidden, target, lenseq, emb, w_ih, w_hh, b_ih, b_hh, w_out, b_out):
    assert not np.asarray(b_ih).any() and not np.asarray(b_hh).any() and not np.asarray(b_out).any(), (
        "nonzero biases not supported by this kernel build"
    )
    target = np.asarray(target)
    X = np.empty((SEQ, B), dtype=np.int64)
    X[0] = SOS_ID
    X[1:] = target[:SEQ - 1]
    done = ((X == EOS_ID) | (X == PAD_ID)).astype(np.float32)  # [SEQ, B]
    emb = np.asarray(emb, dtype=np.float32)
    E = emb[X.reshape(-1)]                                     # [2048, 512]
    eT = np.ascontiguousarray(E.T).astype(BF16)                # [512, 2048]
    h0 = np.asarray(hidden, dtype=np.float32)[0]               # [32, 1024]
    hT0 = np.ascontiguousarray(h0.T).astype(BF16)              # [1024, 32]
    done_s = np.ascontiguousarray(done.T)                      # [B, SEQ]
    done_row = done.reshape(ROWS)                              # [2048]
    done_pb = np.ascontiguousarray(done_row.reshape(MCH, 128).T)  # [128, 16]
    live_pb = np.ascontiguousarray(1.0 - done_pb)
    w_ih = np.asarray(w_ih, dtype=np.float32)
    w_hh = np.asarray(w_hh, dtype=np.float32)
    w_out = np.asarray(w_out, dtype=np.float32)

    in_maps = []
    for c in range(NC):
        rows = np.r_[c * HSL:(c + 1) * HSL,
                     HID + c * HSL:HID + (c + 1) * HSL,
                     2 * HID + c * HSL:2 * HID + (c + 1) * HSL]
        wihT = np.ascontiguousarray(w_ih[rows].T).astype(BF16)     # [512, 384]
        whhT = np.ascontiguousarray(w_hh[rows].T).astype(BF16)     # [1024, 384]
        woutT = np.ascontiguousarray(w_out[c * VS:(c + 1) * VS].T).astype(BF16)  # [1024,4000]
        h0own = np.ascontiguousarray(h0[:, c * HSL:(c + 1) * HSL])
        pad_pb = done_pb if c == 0 else np.zeros_like(done_pb)
        in_maps.append({
            "eT": eT, "wihT": wihT, "whhT": whhT, "woutT": woutT,
            "hT0": hT0, "h0own": h0own, "doneS": done_s,
            "livePB": live_pb, "padPB": np.ascontiguousarray(pad_pb),
        })
    return in_maps


def kernel(hidden, target, lenseq, emb, w_ih, w_hh, b_ih, b_hh, w_out, b_out):
    from concourse.bass_utils import run_bass_kernel_spmd

    in_maps = _host_prep(hidden, target, lenseq, emb, w_ih, w_hh, b_ih, b_hh,
                         w_out, b_out)
    if "nc" not in _CACHE:
        _CACHE["nc"] = _build()
    res = run_bass_kernel_spmd(_CACHE["nc"], in_maps, core_ids=list(range(NC)))
    outs = [r["out"] for r in res.results]                     # each [2048, 4000]
    full = np.concatenate(outs, axis=1).reshape(SEQ, B, VOCAB)
    return full[:int(lenseq)]

